# revision 8
# baseline (speedup 1.0000x reference)
"""Trainium2 Bass kernel for nn_BLoss: loss = mean_i(max(0, sum_j B[i,j] - 1)).

Data-parallel over 8 NeuronCores: each core takes a [1024, 16384] row shard,
streams it through SBUF, reduces each row (free-dim reduce on VectorE),
applies the hinge on ScalarE, collapses partitions with a ones-matmul on
TensorE, and writes one partial scalar. The host sums the 8 partials and
divides by the global batch.
"""

import numpy as np
from contextlib import ExitStack

import concourse.bass as bass
import concourse.tile as tile
from concourse import bacc, mybir
from concourse.bass_utils import run_bass_kernel_spmd

N_CORES = 8
ROWS, COLS = 8192, 16384
SHARD_ROWS = ROWS // N_CORES  # 1024
P = 128                       # SBUF partitions
N_RT = SHARD_ROWS // P        # 8 row tiles per core
CHUNK = 8192                  # columns per DMA chunk (4 MiB per transfer)
N_CHUNKS = COLS // CHUNK
PENALTY_B = 1.0

_PROGRAM = None


def _build_program() -> bass.Bass:
    nc = bacc.Bacc("TRN2", target_bir_lowering=False, debug=False)
    B = nc.declare_dram_parameter(
        "B", [SHARD_ROWS, COLS], mybir.dt.float32, isOutput=False
    )
    out = nc.declare_dram_parameter("out", [1, 1], mybir.dt.float32, isOutput=True)

    with ExitStack() as ctx:
        tc = ctx.enter_context(tile.TileContext(nc))
        data = ctx.enter_context(tc.tile_pool(name="data", bufs=4))
        stats = ctx.enter_context(tc.tile_pool(name="stats", bufs=1))
        psum = ctx.enter_context(tc.tile_pool(name="psum", bufs=1, space="PSUM"))

        partials = stats.tile([P, N_RT * N_CHUNKS], mybir.dt.float32)
        ones = stats.tile([P, 1], mybir.dt.float32)
        nc.vector.memset(ones[:], 1.0)
        neg1 = stats.tile([P, 1], mybir.dt.float32)
        nc.vector.memset(neg1[:], -1.0)

        for r in range(N_RT):
            for c in range(N_CHUNKS):
                t = data.tile([P, CHUNK], mybir.dt.float32)
                nc.gpsimd.dma_start(
                    t[:], B[r * P : (r + 1) * P, c * CHUNK : (c + 1) * CHUNK]
                )
                i = r * N_CHUNKS + c
                nc.vector.reduce_sum(
                    partials[:, i : i + 1], t[:], axis=mybir.AxisListType.X
                )

        # Per-row sums across column chunks, then hinge = relu(rowsum - 1).
        rowsums = stats.tile([P, N_RT], mybir.dt.float32)
        nc.vector.reduce_sum(
            rowsums[:],
            partials[:].rearrange("p (r c) -> p r c", c=N_CHUNKS),
            axis=mybir.AxisListType.X,
        )
        hinges = stats.tile([P, N_RT], mybir.dt.float32)
        nc.scalar.activation(
            hinges[:], rowsums[:], mybir.ActivationFunctionType.Relu, bias=neg1[:]
        )
        hsum = stats.tile([P, 1], mybir.dt.float32)
        nc.vector.reduce_sum(hsum[:], hinges[:], axis=mybir.AxisListType.X)

        # Partition-dim reduction: [1,1] = ones[128,1].T @ hsum[128,1].
        acc = psum.tile([1, 1], mybir.dt.float32)
        nc.tensor.matmul(acc[:], ones[:], hsum[:], start=True, stop=True)
        res = stats.tile([1, 1], mybir.dt.float32)
        nc.scalar.copy(res[:], acc[:])
        nc.sync.dma_start(out[:], res[:])

    nc.compile()
    return nc


def _run(B: np.ndarray, trace: bool = False):
    global _PROGRAM
    if _PROGRAM is None:
        _PROGRAM = _build_program()
    in_maps = [
        {"B": B[i * SHARD_ROWS : (i + 1) * SHARD_ROWS]} for i in range(N_CORES)
    ]
    res = run_bass_kernel_spmd(_PROGRAM, in_maps, list(range(N_CORES)), trace=trace)
    total = sum(np.float64(r["out"][0, 0]) for r in res.results)
    value = np.asarray(np.float32(PENALTY_B * total / ROWS))
    return value, res


def kernel(B: np.ndarray) -> np.ndarray:
    B = np.ascontiguousarray(np.asarray(B, dtype=np.float32))
    assert B.shape == (ROWS, COLS), B.shape
    value, _ = _run(B, trace=False)
    return value
